# revision 1
# baseline (speedup 1.0000x reference)
"""Gaussian basis functions on 8 Trainium2 cores.

out[m] = sum_n w[n] * exp(-0.5 * (p_m - u_n)^T Sigma_n^{-1} (p_m - u_n))

Host precomputes per-Gaussian inverse covariance (O(N) tiny work), folds
log(w) into the exponent constant, and builds 13-dim feature vectors so the
exponent becomes a single K=13 matmul:
  exponent[m,n] = Paug[m,:] . Naug[n,:]
  Paug[m] = [pp(9), p(3), 1]        (per point)
  Naug[n] = [-0.5*A9, b, -0.5*uu + log w]   (per gaussian)
Each core gets M/8 = 8192 points (data parallel), N params replicated.
Device: 64 m-tiles x 4 n-chunks of matmul(13x128x512) -> PSUM, then
scalar-engine Exp with accum_out (free-axis sum), vector reduce of the 4
partials, one DMA of the (128,64) result.
"""

import sys

sys.path.insert(0, "/opt/trn_rl_repo")

import numpy as np

M, N, NCORES = 65536, 2048, 8
MC = M // NCORES  # 8192 points per core
K = 13
MT = 128  # points per m-tile (PSUM partitions)
NT = 512  # gaussians per n-chunk (PSUM bank free size, f32)
NMT = MC // MT  # 64
NNT = N // NT  # 4
EPS_QUAT = 1e-8
EPS_COV = 1e-6

_CACHE = {}


def _build_bass():
    from concourse import bacc, tile
    import concourse.mybir as mybir
    from concourse.bass import MemorySpace

    f32 = mybir.dt.float32
    nc = bacc.Bacc(None, target_bir_lowering=False, debug=False)

    paugt_d = nc.dram_tensor("paugt", [K, MC], f32, kind="ExternalInput")
    naug_d = nc.dram_tensor("naug", [K, N], f32, kind="ExternalInput")
    out_d = nc.dram_tensor("out", [MT, NMT], f32, kind="ExternalOutput")

    with tile.TileContext(nc) as tc:
        with (
            tc.tile_pool(name="const", bufs=1) as cpool,
            tc.tile_pool(name="work", bufs=4) as wpool,
            tc.tile_pool(name="psum", bufs=6, space=MemorySpace.PSUM) as ppool,
        ):
            paugt = cpool.tile([K, MC], f32)
            naug = cpool.tile([K, N], f32)
            acc = cpool.tile([MT, NMT], f32)
            nc.sync.dma_start(out=paugt[:], in_=paugt_d[:])
            nc.sync.dma_start(out=naug[:], in_=naug_d[:])

            for t in range(NMT):
                partial = wpool.tile([MT, NNT], f32, tag="partial")
                for j in range(NNT):
                    ps = ppool.tile([MT, NT], f32, tag="ps")
                    nc.tensor.matmul(
                        ps[:],
                        paugt[:, t * MT : (t + 1) * MT],
                        naug[:, j * NT : (j + 1) * NT],
                        start=True,
                        stop=True,
                    )
                    scratch = wpool.tile([MT, NT], f32, tag="scratch")
                    nc.scalar.activation(
                        scratch[:],
                        ps[:],
                        mybir.ActivationFunctionType.Exp,
                        accum_out=partial[:, j : j + 1],
                    )
                nc.vector.reduce_sum(
                    acc[:, t : t + 1], partial[:], axis=mybir.AxisListType.X
                )
            nc.sync.dma_start(out=out_d[:], in_=acc[:])

    nc.compile()
    return nc


def _preprocess(points, positions, log_scales, rotations, weights):
    p64 = points.astype(np.float64)
    pos = positions.astype(np.float64)
    s = np.exp(log_scales.astype(np.float64))
    q = rotations.astype(np.float64)
    q = q / (np.linalg.norm(q, axis=1, keepdims=True) + EPS_QUAT)
    w, x, y, z = q[:, 0], q[:, 1], q[:, 2], q[:, 3]
    R = np.empty((q.shape[0], 3, 3), np.float64)
    R[:, 0, 0] = 1 - 2 * (y * y + z * z)
    R[:, 0, 1] = 2 * (x * y - z * w)
    R[:, 0, 2] = 2 * (x * z + y * w)
    R[:, 1, 0] = 2 * (x * y + z * w)
    R[:, 1, 1] = 1 - 2 * (x * x + z * z)
    R[:, 1, 2] = 2 * (y * z - x * w)
    R[:, 2, 0] = 2 * (x * z - y * w)
    R[:, 2, 1] = 2 * (y * z + x * w)
    R[:, 2, 2] = 1 - 2 * (x * x + y * y)
    cov = np.einsum("nij,nj,nkj->nik", R, s * s, R) + EPS_COV * np.eye(3)
    A = np.linalg.inv(cov)
    A9 = A.reshape(-1, 9)
    b = np.einsum("nij,nj->ni", A, pos)
    uu = np.einsum("ni,ni->n", pos, b)
    logw = np.log(np.maximum(weights.astype(np.float64), 1e-300))
    naug = np.concatenate(
        [-0.5 * A9, b, (-0.5 * uu + logw)[:, None]], axis=1
    ).T  # (13, N)

    PP = (p64[:, :, None] * p64[:, None, :]).reshape(-1, 9)
    paugt = np.concatenate([PP, p64, np.ones((p64.shape[0], 1))], axis=1).T  # (13, M)
    return (
        np.ascontiguousarray(paugt).astype(np.float32),
        np.ascontiguousarray(naug).astype(np.float32),
    )


def kernel(points, positions, log_scales, rotations, weights):
    from concourse import bass_utils

    if "nc" not in _CACHE:
        _CACHE["nc"] = _build_bass()
    nc = _CACHE["nc"]

    paugt, naug = _preprocess(points, positions, log_scales, rotations, weights)
    in_maps = [
        {
            "paugt": np.ascontiguousarray(paugt[:, c * MC : (c + 1) * MC]),
            "naug": naug,
        }
        for c in range(NCORES)
    ]
    res = bass_utils.run_bass_kernel_spmd(nc, in_maps, list(range(NCORES)))
    outs = [r["out"].T.reshape(-1) for r in res.results]
    return np.concatenate(outs).astype(np.float32)


# revision 2
# speedup vs baseline: 2.0689x; 2.0689x over previous
"""Gaussian basis functions on 8 Trainium2 cores.

out[m] = sum_n w[n] * exp(-0.5 * (p_m - u_n)^T Sigma_n^{-1} (p_m - u_n))

Host precomputes per-Gaussian inverse covariance (O(N) tiny work), folds
log(w) into the exponent constant, and builds 13-dim feature vectors so the
exponent becomes a single K=13 matmul:
  exponent[m,n] = Paug[m,:] . Naug[n,:]
  Paug[m] = [pp(9), p(3), 1]        (per point)
  Naug[n] = [-0.5*A9, b, -0.5*uu + log w]   (per gaussian)
Each core gets M/8 = 8192 points (data parallel), N params replicated.
Device: 64 m-tiles x 4 n-chunks of matmul(13x128x512) -> PSUM, then
scalar-engine Exp with accum_out (free-axis sum), vector reduce of the 4
partials, one DMA of the (128,64) result.
"""

import sys

sys.path.insert(0, "/opt/trn_rl_repo")

import numpy as np

M, N, NCORES = 65536, 2048, 8
MC = M // NCORES  # 8192 points per core
K = 13
MT = 128  # points per m-tile (PSUM partitions)
NT = 512  # gaussians per n-chunk (PSUM bank free size, f32)
NMT = MC // MT  # 64
NNT = N // NT  # 4
EPS_QUAT = 1e-8
EPS_COV = 1e-6

_CACHE = {}


def _build_bass():
    from concourse import bacc, tile
    import concourse.mybir as mybir
    from concourse.bass import MemorySpace

    f32 = mybir.dt.float32
    nc = bacc.Bacc(None, target_bir_lowering=False, debug=False)

    paugt_d = nc.dram_tensor("paugt", [K, MC], f32, kind="ExternalInput")
    naug_d = nc.dram_tensor("naug", [K, N], f32, kind="ExternalInput")
    out_d = nc.dram_tensor("out", [MT, NMT], f32, kind="ExternalOutput")

    with tile.TileContext(nc) as tc:
        with (
            tc.tile_pool(name="const", bufs=1) as cpool,
            tc.tile_pool(name="work", bufs=4) as wpool,
            tc.tile_pool(name="psum", bufs=6, space=MemorySpace.PSUM) as ppool,
        ):
            paugt = cpool.tile([K, MC], f32)
            naug = cpool.tile([K, N], f32)
            acc = cpool.tile([MT, NMT], f32)
            nc.sync.dma_start(out=paugt[:], in_=paugt_d[:])
            nc.sync.dma_start(out=naug[:], in_=naug_d[:])

            for t in range(NMT):
                partial = wpool.tile([MT, NNT], f32, tag="partial")
                for j in range(NNT):
                    ps = ppool.tile([MT, NT], f32, tag="ps")
                    nc.tensor.matmul(
                        ps[:],
                        paugt[:, t * MT : (t + 1) * MT],
                        naug[:, j * NT : (j + 1) * NT],
                        start=True,
                        stop=True,
                    )
                    scratch = wpool.tile([MT, NT], f32, tag="scratch")
                    nc.scalar.activation(
                        scratch[:],
                        ps[:],
                        mybir.ActivationFunctionType.Exp,
                        accum_out=partial[:, j : j + 1],
                    )
                nc.vector.reduce_sum(
                    acc[:, t : t + 1], partial[:], axis=mybir.AxisListType.X
                )
            nc.sync.dma_start(out=out_d[:], in_=acc[:])

    nc.compile()
    return nc


def _preprocess(points, positions, log_scales, rotations, weights):
    p64 = points.astype(np.float64)
    pos = positions.astype(np.float64)
    s = np.exp(log_scales.astype(np.float64))
    q = rotations.astype(np.float64)
    q = q / (np.linalg.norm(q, axis=1, keepdims=True) + EPS_QUAT)
    w, x, y, z = q[:, 0], q[:, 1], q[:, 2], q[:, 3]
    R = np.empty((q.shape[0], 3, 3), np.float64)
    R[:, 0, 0] = 1 - 2 * (y * y + z * z)
    R[:, 0, 1] = 2 * (x * y - z * w)
    R[:, 0, 2] = 2 * (x * z + y * w)
    R[:, 1, 0] = 2 * (x * y + z * w)
    R[:, 1, 1] = 1 - 2 * (x * x + z * z)
    R[:, 1, 2] = 2 * (y * z - x * w)
    R[:, 2, 0] = 2 * (x * z - y * w)
    R[:, 2, 1] = 2 * (y * z + x * w)
    R[:, 2, 2] = 1 - 2 * (x * x + y * y)
    cov = np.einsum("nij,nj,nkj->nik", R, s * s, R) + EPS_COV * np.eye(3)
    A = np.linalg.inv(cov)
    A9 = A.reshape(-1, 9)
    b = np.einsum("nij,nj->ni", A, pos)
    uu = np.einsum("ni,ni->n", pos, b)
    logw = np.log(np.maximum(weights.astype(np.float64), 1e-300))
    naug = np.concatenate(
        [-0.5 * A9, b, (-0.5 * uu + logw)[:, None]], axis=1
    ).T  # (13, N)

    pT = np.ascontiguousarray(points.astype(np.float32).T)  # (3, M)
    paugt = np.empty((K, pT.shape[1]), np.float32)
    for i in range(3):
        for j in range(3):
            np.multiply(pT[i], pT[j], out=paugt[i * 3 + j])
    paugt[9:12] = pT
    paugt[12] = 1.0
    return paugt, np.ascontiguousarray(naug).astype(np.float32)


def _get_runner():
    """Build the jitted shard_map executable once (mirrors
    bass2jax.run_bass_via_pjrt, which re-traces on every call)."""
    if "runner" in _CACHE:
        return _CACHE["runner"]
    import jax
    from concourse import bass2jax
    from jax.sharding import Mesh, PartitionSpec
    from jax.experimental.shard_map import shard_map
    import concourse.mybir as mybir

    nc = _CACHE.get("nc") or _build_bass()
    _CACHE["nc"] = nc
    bass2jax.install_neuronx_cc_hook()

    partition_name = nc.partition_id_tensor.name if nc.partition_id_tensor else None
    in_names, out_names, out_avals, zero_shapes = [], [], [], []
    for alloc in nc.m.functions[0].allocations:
        if not isinstance(alloc, mybir.MemoryLocationSet):
            continue
        name = alloc.memorylocations[0].name
        if alloc.kind == "ExternalInput":
            if name != partition_name:
                in_names.append(name)
        elif alloc.kind == "ExternalOutput":
            out_names.append(name)
            shape = tuple(alloc.tensor_shape)
            dtype = mybir.dt.np(alloc.dtype)
            out_avals.append(jax.core.ShapedArray(shape, dtype))
            zero_shapes.append((shape, dtype))
    n_params = len(in_names)
    all_names = list(in_names) + out_names
    if partition_name is not None:
        all_names.append(partition_name)

    def _body(*args):
        operands = list(args)
        if partition_name is not None:
            operands.append(bass2jax.partition_id_tensor())
        return tuple(
            bass2jax._bass_exec_p.bind(
                *operands,
                out_avals=tuple(out_avals),
                in_names=tuple(all_names),
                out_names=tuple(out_names),
                lowering_input_output_aliases=(),
                sim_require_finite=True,
                sim_require_nnan=True,
                nc=nc,
            )
        )

    devices = jax.devices()[:NCORES]
    mesh = Mesh(np.asarray(devices), ("core",))
    n_outs = len(out_names)
    sharded = jax.jit(
        shard_map(
            _body,
            mesh=mesh,
            in_specs=(PartitionSpec("core"),) * (n_params + n_outs),
            out_specs=(PartitionSpec("core"),) * n_outs,
            check_rep=False,
        ),
        donate_argnums=tuple(range(n_params, n_params + n_outs)),
        keep_unused=True,
    )
    _CACHE["runner"] = (sharded, in_names, zero_shapes)
    return _CACHE["runner"]


def kernel(points, positions, log_scales, rotations, weights):
    sharded, in_names, zero_shapes = _get_runner()

    paugt, naug = _preprocess(points, positions, log_scales, rotations, weights)
    # concat per-core shards on axis 0: (8*13, MC) / (8*13, N)
    inputs_by_name = {
        "paugt": np.ascontiguousarray(
            paugt.reshape(K, NCORES, MC).transpose(1, 0, 2).reshape(NCORES * K, MC)
        ),
        "naug": np.tile(naug, (NCORES, 1)),
    }
    concat_in = [inputs_by_name[n] for n in in_names]
    concat_zeros = [
        np.zeros((NCORES * s[0], *s[1:]), d) for (s, d) in zero_shapes
    ]
    out_arrs = sharded(*concat_in, *concat_zeros)
    arr = np.asarray(out_arrs[0]).reshape(NCORES, MT, NMT)
    # out[c*MC + t*MT + p] = arr[c, p, t]
    return np.ascontiguousarray(arr.transpose(0, 2, 1)).reshape(-1).astype(np.float32)
